# revision 8
# baseline (speedup 1.0000x reference)
"""Trainium2 Bass kernel for MiniSelfAttention, v3 (all-bf16).

Shapes (full problem): x (4, 2048, 1024), Wq/Wk/Wv/Wo (1024, 1024), bo (1024,).
H=16 heads, D=64. out = softmax(q k^T / 8) v  projected by Wo.

Sharding across 8 cores: core c -> batch b = c//2, head-group g = c%2
(8 heads = 512 features per group).  Each core computes a partial output
projection; host sums the two partials per batch and adds the bias.

v3 structure (per core, T=2048, V=1024, F=512, D=64, H=8):
  - p-outer attention; head-pair p's windows run while the NEXT pair's q/k
    projection chains are interleaved into the PE stream; the v projection
    chains are interleaved into the first window; out-projection chains
    into p=3's windows.  All engines stay busy concurrently.
  - scores: two K=64 row-quadrant matmuls (concurrent on HW).
  - ctx half0: split the s-contraction into rows 0:64 / 64:128 as two
    row-quadrant matmuls accumulating into separate PSUM banks (pA, pB) --
    they run concurrently like the scores pair; summed during the
    normalization.  ctx half1: one full-row matmul (pC).  PSUM budget:
    sc 4 + pA/pB/pC 3 + mm-ring 1 = 8 banks.
  - exp on ScalarE ([128,1024] per s-tile, both heads in one op); for
    DVE_PAIRS s-pairs, VectorE computes a Schraudolph exp2 (one
    tensor_scalar -> uint16, bitcast bf16) to relieve ScalarE.
  - ctx matmuls deferred one s-tile behind scores so the PE FIFO never
    stalls on the exp.
"""

import sys

sys.path.insert(0, "/opt/trn_rl_repo")

import numpy as np

import concourse.bacc as bacc
import concourse.mybir as mybir
from concourse import tile
from concourse.bass_utils import run_bass_kernel_spmd

F32 = mybir.dt.float32
BF16 = mybir.dt.bfloat16
U16 = mybir.dt.uint16
AF = mybir.ActivationFunctionType
ALU = mybir.AluOpType

DIM = 1024
HEADS = 16
D = 64
N_CORES = 8

LOG2E = 1.4426950408889634
# Schraudolph constants: bf16 bits of e^(x/8) = 2^(x*log2e/8)
SCH_A = 128.0 * LOG2E / 8.0
SCH_B = 127.0 * 128.0 - 0.5 * 128.0 * 0.0579

DVE_PAIRS = ()  # s-pairs whose exp runs on the Vector engine


def build_nc(T=2048, V=DIM, F=512):
    H = F // D                 # heads per core (8)
    P = H // 2                 # head pairs (4)
    KC = V // 128              # contraction chunks for projections (8)
    NT = T // 128              # s tiles (16)
    TQ = 512                   # q block (free dim of scores)
    NJ = T // TQ               # q blocks (4)
    KF = F // 128              # ctx feature chunks (4)

    nc = bacc.Bacc(trn_type="TRN2")
    xT = nc.dram_tensor("xT", [V, T], BF16, kind="ExternalInput")
    wqT = nc.dram_tensor("wqT", [V, F], BF16, kind="ExternalInput")
    wkT = nc.dram_tensor("wkT", [V, F], BF16, kind="ExternalInput")
    wvT = nc.dram_tensor("wvT", [V, F], BF16, kind="ExternalInput")
    woT = nc.dram_tensor("woT", [F, V], BF16, kind="ExternalInput")
    out = nc.dram_tensor("out", [T, V], F32, kind="ExternalOutput")

    with tile.TileContext(nc) as tc:
        with (
            tc.tile_pool(name="persist", bufs=1) as pp,
            tc.tile_pool(name="expp", bufs=1) as pe,
            tc.tile_pool(name="rz", bufs=1) as prz,
            tc.tile_pool(name="ps", bufs=1, space="PSUM") as psp,
        ):
            # Persistent SBUF tensors.
            qT = [pp.tile([128, T], BF16, tag=f"qT{m}", name=f"qT{m}") for m in range(P)]
            kT = [pp.tile([128, T], BF16, tag=f"kT{m}", name=f"kT{m}") for m in range(P)]
            ctxT = [pp.tile([128, T], BF16, tag=f"cT{m}", name=f"cT{m}") for m in range(P)]
            # v per s-tile: [s(128), head, 80]; cols 0:64 = v, col 64 = ones (Z)
            vaug = [pp.tile([128, H, 80], BF16, tag=f"va{t}", name=f"va{t}") for t in range(NT)]
            xTs = [pp.tile([128, T], BF16, tag=f"xT{k}", name=f"xTs{k}") for k in range(KC)]

            def alloc_w(nm):
                return [
                    pp.tile([128, F], BF16, tag=f"{nm}{k}", name="w")
                    for k in range(KC)
                ]

            wvs = alloc_w("wv")
            wqs = alloc_w("wq")
            wks = alloc_w("wk")
            # DMA order follows first use: x + q/k weights (interleaved per
            # chunk) feed the qk chains; v weights next; Wo last.
            for k in range(KC):
                nc.sync.dma_start(xTs[k][:], xT[128 * k : 128 * (k + 1), :])
                nc.scalar.dma_start(wqs[k][:], wqT[128 * k : 128 * (k + 1), :])
                nc.gpsimd.dma_start(wks[k][:], wkT[128 * k : 128 * (k + 1), :])
                nc.gpsimd.dma_start(wvs[k][:], wvT[128 * k : 128 * (k + 1), :])
            wos = []
            for k in range(KF):
                w = pp.tile([128, V], BF16, tag=f"wo{k}", name=f"wo{k}")
                nc.sync.dma_start(w[:], woT[128 * k : 128 * (k + 1), :])
                wos.append(w)

            # ---- PE chain pieces (also used as window fillers) ----

            def v_chain(t):
                nc.vector.memset(vaug[t][:, :, 64:65], 1.0)
                ps = psp.tile([128, 512], F32, tag="mm", bufs=2, name="psmm")
                for k in range(KC):
                    nc.tensor.matmul(
                        ps[:],
                        xTs[k][:, 128 * t : 128 * (t + 1)],
                        wvs[k][:],
                        start=(k == 0),
                        stop=(k == KC - 1),
                    )
                nc.vector.tensor_copy(
                    vaug[t][:, :, 0:64],
                    ps[:].rearrange("p (h d) -> p h d", h=H),
                )

            def qk_chain(p, w, n):
                ws, dst = ((wqs, qT) if w == 0 else (wks, kT))
                ps = psp.tile([128, 512], F32, tag="mm", bufs=2, name="psmm")
                for k in range(KC):
                    nc.tensor.matmul(
                        ps[:],
                        ws[k][:, 128 * p : 128 * (p + 1)],
                        xTs[k][:, 512 * n : 512 * (n + 1)],
                        start=(k == 0),
                        stop=(k == KC - 1),
                    )
                nc.vector.tensor_copy(dst[p][:, 512 * n : 512 * (n + 1)], ps[:])

            def out_chain(j, ti, n):
                t = 4 * j + ti
                ps = psp.tile([128, 512], F32, tag="mm", bufs=2, name="psmm")
                for k in range(KF):
                    nc.tensor.matmul(
                        ps[:],
                        ctxT[k][:, 128 * t : 128 * (t + 1)],
                        wos[k][:, 512 * n : 512 * (n + 1)],
                        start=(k == 0),
                        stop=(k == KF - 1),
                    )
                ot = pp.tile([128, 512], F32, tag="ot", bufs=4, name="ot")
                nc.vector.tensor_copy(ot[:], ps[:])
                nc.sync.dma_start(
                    out[128 * t : 128 * (t + 1), 512 * n : 512 * (n + 1)], ot[:]
                )

            # ---- attention pieces ----

            def scores(j, p, s):
                sc = psp.tile([128, 2 * TQ], F32, tag="sc", bufs=2, name="sc")
                for half in range(2):
                    lo, hi = 64 * half, 64 * half + 64
                    nc.tensor.matmul(
                        sc[:, TQ * half : TQ * (half + 1)],
                        kT[p][lo:hi, 128 * s : 128 * (s + 1)],
                        qT[p][lo:hi, TQ * j : TQ * (j + 1)],
                        tile_position=(lo, 0),
                    )
                return sc

            def ctx_mms(p, s, pA, pC, e, first, last):
                h0, h1 = 2 * p, 2 * p + 1
                nc.tensor.matmul(
                    pA[0:65, :], vaug[s][:, h0, 0:65], e[:, 0, :],
                    start=first, stop=last,
                )
                nc.tensor.matmul(
                    pC[0:65, :], vaug[s][:, h1, 0:65], e[:, 1, :],
                    start=first, stop=last,
                )

            def normalize(j, p, pA, pC):
                for half, pcx in ((0, pA), (1, pC)):
                    lo, hi = 64 * half, 64 * half + 64
                    zs = prz.tile([1, TQ], F32, tag=f"z{half}", bufs=2, name="zs")
                    nc.vector.tensor_copy(zs[:], pcx[64:65, :])
                    r1 = prz.tile([1, TQ], F32, tag=f"r{half}", bufs=2, name="r1")
                    nc.vector.reciprocal_approx_fast(r1[:], zs[:])
                    b1 = prz.tile([64, TQ], F32, tag=f"b{half}", bufs=2, name="b1")
                    nc.gpsimd.partition_broadcast(b1[:], r1[:])
                    nc.vector.tensor_mul(
                        ctxT[p][lo:hi, TQ * j : TQ * (j + 1)], pcx[0:64, :], b1[:]
                    )

            def window(j, p, fillers):
                fi = 0
                pA = psp.tile([128, TQ], F32, tag="cxa", bufs=1, name="pA")
                pC = psp.tile([128, TQ], F32, tag="cxc", bufs=1, name="pC")
                pend = []
                for sp2 in range(NT // 2):
                    # both s-tiles' score pairs back-to-back: the second
                    # pair's LDW hides behind the first pair's row-64 MM
                    for ab in range(2):
                        s = 2 * sp2 + ab
                        e = pe.tile([128, 2, TQ], BF16, tag="e", bufs=5, name="e")
                        sc = scores(j, p, s)
                        if sp2 in DVE_PAIRS:
                            nc.vector.tensor_scalar(
                                e[:].bitcast(U16), sc[:], SCH_A, SCH_B,
                                ALU.mult, ALU.add,
                            )
                        else:
                            nc.scalar.activation(e[:], sc[:], AF.Exp, scale=1.0 / 8.0)
                        pend.append((p, s, pA, pC, e, s == 0, s == NT - 1))
                    # filler chains between the scores block and deferred ctx
                    nfill = 2 if len(fillers) > 8 else 1
                    for _ in range(nfill):
                        if fi < len(fillers):
                            f, args = fillers[fi]
                            f(*args)
                            fi += 1
                    while len(pend) > 2:
                        ctx_mms(*pend.pop(0))
                for pd in pend:
                    ctx_mms(*pd)
                while fi < len(fillers):
                    f, args = fillers[fi]
                    f(*args)
                    fi += 1
                normalize(j, p, pA, pC)

            # ---- emission ----
            for n in range(NJ):
                qk_chain(0, 0, n)
                qk_chain(0, 1, n)
            qk1 = [(qk_chain, (1, w, n)) for n in range(NJ) for w in range(2)]
            for p in range(P):
                for j in range(NJ):
                    if p == 0:
                        fillers = (
                            [(v_chain, (t,)) for t in range(NT)]
                            if j == 0
                            else qk1[3 * j - 3 : 3 * j]
                        )
                    elif p < 3:
                        fillers = [(qk_chain, (p + 1, w, j)) for w in range(2)]
                    else:
                        fillers = (
                            [(out_chain, (j - 1, ti, n)) for ti in range(4) for n in range(2)]
                            if j > 0
                            else []
                        )
                    window(j, p, fillers)
            for ti in range(4):
                for n in range(2):
                    out_chain(NJ - 1, ti, n)

    nc.compile()
    return nc


_NC_CACHE = {}


def _get_nc(T=2048, V=DIM, F=512):
    key = (T, V, F)
    if key not in _NC_CACHE:
        _NC_CACHE[key] = build_nc(T, V, F)
    return _NC_CACHE[key]


def make_in_maps(x, Wq, Wk, Wv, Wo):
    np_bf16 = mybir.dt.np(BF16)
    B = x.shape[0]
    F = Wq.shape[0] // 2
    in_maps = []
    for c in range(N_CORES):
        b, g = divmod(c, 2)
        rows = slice(g * F, (g + 1) * F)
        in_maps.append(
            {
                "xT": np.ascontiguousarray(x[b].T).astype(np_bf16),
                "wqT": np.ascontiguousarray(Wq[rows].T).astype(np_bf16),
                "wkT": np.ascontiguousarray(Wk[rows].T).astype(np_bf16),
                "wvT": np.ascontiguousarray(Wv[rows].T).astype(np_bf16),
                "woT": np.ascontiguousarray(Wo[:, rows].T).astype(np_bf16),
            }
        )
    return in_maps


def kernel(x, Wq, Wk, Wv, Wo, bo, trace=False):
    x = np.asarray(x, np.float32)
    B, T, V = x.shape
    nc = _get_nc(T=T, V=V, F=V // 2)
    in_maps = make_in_maps(
        x,
        np.asarray(Wq, np.float32),
        np.asarray(Wk, np.float32),
        np.asarray(Wv, np.float32),
        np.asarray(Wo, np.float32),
    )
    res = run_bass_kernel_spmd(nc, in_maps, core_ids=list(range(N_CORES)), trace=trace)
    outs = [r["out"] for r in res.results]
    full = np.empty((B, T, V), np.float32)
    for b in range(B):
        full[b] = outs[2 * b] + outs[2 * b + 1] + np.asarray(bo, np.float32)
    if trace:
        kernel.last_exec_time_ns = res.exec_time_ns
        kernel.last_results = res
    return full
